# revision 10
# baseline (speedup 1.0000x reference)
"""EnhancedRGCN (3-layer GAT) Trainium2 kernel, 8-core SPMD.

Sharding: destination nodes across 8 cores. Host builds a static padded-CSR
(dst-degree-sorted, windows of 128 dst nodes, groups of 4 windows sharing a
padded width). Per layer: node phase computes fp16 table rows
[h | a_s-pair | a_d-pair] = act(prev) @ Wbig via PE, one transposed-AP DMA
writes the shard table, AllGather exchanges shards, edge phase gathers rows
per group with a single batched indirect DMA (flat-table AP, descriptors
collapse to 128/call), then DVE/ACT run the segment softmax + weighted
aggregation in fp16 with a pair-interleaved head layout (keeps tensor_tensor
in 2x mode). Softmax max-subtraction is skipped (bounded logits); pad slots
gather a dedicated table row with h=0, a_s=-3000 so exp() underflows to 0.
"""

import sys

sys.path.insert(0, "/opt/trn_rl_repo")

import numpy as np

from concourse import bass, bacc, mybir, tile
from concourse.bass_utils import run_bass_kernel_spmd
from concourse.masks import make_identity

NC = 8
P = 128
GS = 4  # windows per group
F32 = mybir.dt.float32
F16 = mybir.dt.float16
I32 = mybir.dt.int32
ALU = mybir.AluOpType
ACTF = mybir.ActivationFunctionType

N_NODES = 100000
NPC = (N_NODES + NC - 1) // NC          # 12500
NW = (NPC + P - 1) // P                 # 98
NP = NW * P                             # 12544
TBL = NC * NP
NG = (NW + GS - 1) // GS                # 25 groups (24x4 + 1x2)
PAD_ROW = NP - 1                        # core 0's rank 12543, never used by data


def _host_prep(edge_index):
    src = np.asarray(edge_index[0], dtype=np.int64)
    dst = np.asarray(edge_index[1], dtype=np.int64)

    table_pos = np.empty(N_NODES, dtype=np.int64)
    perms = []
    dws = np.zeros((NC, NW), dtype=np.int64)
    cores = []
    for c in range(NC):
        lo, hi = c * NPC, min((c + 1) * NPC, N_NODES)
        n_loc = hi - lo
        emask = (dst >= lo) & (dst < hi)
        e_src, e_dst = src[emask], dst[emask] - lo
        deg = np.bincount(e_dst, minlength=n_loc)
        order = np.argsort(-deg, kind="stable")
        perms.append(order + lo)
        table_pos[order + lo] = c * NP + np.arange(n_loc)
        rank_of_local = np.empty(n_loc, dtype=np.int64)
        rank_of_local[order] = np.arange(n_loc)
        e_rank = rank_of_local[e_dst]
        deg_r = np.bincount(e_rank, minlength=NP)
        dws[c] = [max(int(deg_r[w * P:(w + 1) * P].max()), 1) for w in range(NW)]
        o = np.argsort(e_rank, kind="stable")
        e_rank_s, e_src_s = e_rank[o], e_src[o]
        slot = np.arange(len(e_rank_s)) - np.concatenate(
            [[0], np.cumsum(deg_r)])[e_rank_s]
        cores.append(dict(n_loc=n_loc, e_rank=e_rank_s, e_src=e_src_s,
                          slot=slot))

    dw_u = dws.max(axis=0)
    # group widths: max over the GS windows in each group (shared program)
    gw = np.array([int(dw_u[g * GS:(g + 1) * GS].max()) for g in range(NG)])
    ngw = np.array([min(GS, NW - g * GS) for g in range(NG)])
    gcol0 = np.concatenate([[0], np.cumsum(ngw * gw)]).astype(int)
    S = int(gcol0[-1])

    for c in range(NC):
        cc = cores[c]
        w_of = cc["e_rank"] // P
        row = cc["e_rank"] % P
        g_of = w_of // GS
        col = gcol0[g_of] + (w_of - g_of * GS) * gw[g_of] + cc["slot"]
        idx_u = np.full((P, S), PAD_ROW * 34, dtype=np.int32)
        idx_u[row, col] = (table_pos[cc["e_src"]] * 34).astype(np.int32)
        cc["idx"] = idx_u
        cc["perm"] = perms[c]
    return cores, gw, ngw, gcol0, S, table_pos


def _interleave_nat(H):
    # natural feature index for each interleaved column j
    if H == 1:
        return np.arange(32)
    il = np.empty(32, dtype=np.int64)
    il[0::2] = np.arange(16)          # head 0
    il[1::2] = 16 + np.arange(16)     # head 1
    return il


def _build_program(gw, ngw, gcol0, S, slopes, inv_ks):
    nc = bacc.Bacc("TRN2", target_bir_lowering=False, debug=False,
                   num_devices=NC, dynamic_dma_scratch_size=65536)

    x_sh = nc.dram_tensor("x_sh", [NW, P, 32], F16, kind="ExternalInput")
    idx_in = nc.dram_tensor("idx_in", [P, S], I32, kind="ExternalInput")
    wb_in = nc.dram_tensor("wb_in", [32, 108], F16, kind="ExternalInput")
    b12_in = nc.dram_tensor("b12_in", [P, 64], F16, kind="ExternalInput")
    b3_in = nc.dram_tensor("b3_in", [P, 32], F32, kind="ExternalInput")
    out_d = nc.dram_tensor("out_d", [NW, P, 32], F32, kind="ExternalOutput")

    tbl_sh = nc.dram_tensor("tbl_sh", [NW, P, 34], F16)
    tbl_full = nc.dram_tensor("tbl_full", [1, TBL * 34], F16,
                              addr_space="Shared")
    import os
    DBG = os.environ.get("K_DEBUG", "") == "1"
    if DBG:
        dbg_xall = nc.dram_tensor("dbg_xall", [P, NW, 32], F16,
                                  kind="ExternalOutput")
        dbg_nodev = nc.dram_tensor("dbg_nodev", [P, NW, 36], F16,
                                   kind="ExternalOutput")
        dbg_G = nc.dram_tensor("dbg_G", [P, 4 * int(gw[0]), 34], F16,
                               kind="ExternalOutput")
        dbg_agg = nc.dram_tensor("dbg_agg", [P, NW, 34], F16,
                                 kind="ExternalOutput")

    with tile.TileContext(nc) as tc:
        with (
            tc.tile_pool(name="res", bufs=1) as res,
            tc.tile_pool(name="xTp", bufs=2) as xTp,
            tc.tile_pool(name="gat", bufs=3) as gat,
            tc.tile_pool(name="ptp", bufs=2, space="PSUM") as ptp,
            tc.tile_pool(name="pmm", bufs=2, space="PSUM") as pmm,
        ):
            ident = res.tile([P, P], F16)
            make_identity(nc, ident[:])
            idx_t = res.tile([P, S], I32)
            nc.sync.dma_start(idx_t[:], idx_in[:])
            wb_t = res.tile([32, 108], F16)
            nc.sync.dma_start(wb_t[:], wb_in[:])
            b12_t = res.tile([P, 64], F16)
            nc.sync.dma_start(b12_t[:], b12_in[:])
            b3_t = res.tile([P, 32], F32)
            nc.sync.dma_start(b3_t[:], b3_in[:])
            xall = res.tile([P, NW, 32], F16)
            nc.sync.dma_start(xall[:], x_sh.ap().transpose([1, 0, 2]))
            tneg = res.tile([P, NW, 32], F16)
            nodev = res.tile([P, NW, 36], F16)
            agg = res.tile([P, NW, 34], F16)
            denf = res.tile([P, NW, 2], F32)
            outf = res.tile([P, NW, 32], F32)

            for l in range(3):
                slope = float(slopes[l])
                # ---------------- node phase ----------------
                if l > 0:
                    bia = b12_t[:, 32 * (l - 1):32 * l].unsqueeze(1) \
                        .to_broadcast([P, NW, 32])
                    nc.vector.tensor_tensor(out=xall[:], in0=agg[:, :, 0:32],
                                            in1=bia, op=ALU.add)
                    nc.vector.tensor_scalar_min(tneg[:], xall[:], 0.0)
                    nc.scalar.activation(tneg[:], tneg[:], ACTF.Exp)
                    nc.vector.tensor_scalar_max(xall[:], xall[:], 0.0)
                    nc.vector.tensor_tensor(out=xall[:], in0=xall[:],
                                            in1=tneg[:], op=ALU.add)
                    nc.vector.tensor_scalar(out=xall[:], in0=xall[:],
                                            scalar1=-1.0, scalar2=3.0,
                                            op0=ALU.add, op1=ALU.min)
                    nc.vector.tensor_scalar_max(xall[:], xall[:], -3.0)
                for g in range(NG):
                    g0, nw = g * GS, int(ngw[g])
                    pt = ptp.tile([32, GS, P], F16, tag="pt")
                    for r in range(nw):
                        nc.tensor.transpose(
                            out=pt[:, r, :],
                            in_=xall[:, g0 + r, :], identity=ident[:])
                    xT = xTp.tile([32, GS, P], F16, tag="xT")
                    nc.vector.tensor_copy(xT[:, 0:nw, :], pt[:, 0:nw, :])
                    mm = pmm.tile([P, GS, 36], F32, tag="mm")
                    for r in range(nw):
                        nc.tensor.matmul(
                            mm[:, r, :], lhsT=xT[:, r, :],
                            rhs=wb_t[:, 36 * l:36 * l + 36],
                            start=True, stop=True)
                    nc.vector.tensor_copy(nodev[:, g0:g0 + nw, :],
                                          mm[:, 0:nw, :])
                # pad rows: h=0, a_s=-3000 (ranks 12512..12543 unused)
                nc.vector.memset(nodev[96:128, NW - 1, 0:32], 0.0)
                nc.vector.memset(nodev[96:128, NW - 1, 32:34], -3000.0)
                if DBG and l == 0:
                    nc.sync.dma_start(dbg_xall.ap(), xall[:])
                    nc.sync.dma_start(dbg_nodev.ap(), nodev[:])
                nc.sync.dma_start(tbl_sh.ap().transpose([1, 0, 2]),
                                  nodev[:, :, 0:34])
                # ---------------- exchange ----------------
                nc.gpsimd.collective_compute(
                    "AllGather", ALU.bypass,
                    replica_groups=[list(range(NC))],
                    ins=[tbl_sh.ap()], outs=[tbl_full.ap()],
                )
                # ---------------- edge phase ----------------
                for g in range(NG):
                    g0, nw, w = g * GS, int(ngw[g]), int(gw[g])
                    c0 = int(gcol0[g])
                    G = gat.tile([P, nw, w, 34], F16, tag="G")
                    Gv = G[:].rearrange("p a b f -> p (a b) f")
                    for cc in range(nw * w):
                        nc.gpsimd.indirect_dma_start(
                            out=Gv[:, cc, :], out_offset=None, in_=tbl_full[:],
                            in_offset=bass.IndirectOffsetOnAxis(
                                ap=idx_t[:, c0 + cc:c0 + cc + 1], axis=1),
                        )
                    if DBG and l == 0 and g == 0:
                        nc.sync.dma_start(dbg_G.ap(), G[:].opt())
                    tv = G[:, :, :, 32:34]
                    nc.vector.tensor_tensor(
                        out=tv, in0=tv,
                        in1=nodev[:, g0:g0 + nw, 34:36].unsqueeze(2)
                            .to_broadcast([P, nw, w, 2]),
                        op=ALU.add)
                    nc.scalar.activation(tv, tv, ACTF.Lrelu, alpha=slope)
                    nc.scalar.activation(tv, tv, ACTF.Exp)
                    for wi in range(nw):
                        nc.vector.tensor_tensor(
                            out=G[:, wi, :, 0:32].rearrange("p w (a b) -> p w a b", b=2),
                            in0=G[:, wi, :, 0:32].rearrange("p w (a b) -> p w a b", b=2),
                            in1=G[:, wi, :, 32:34].unsqueeze(2)
                                .to_broadcast([P, w, 16, 2]),
                            op=ALU.mult)
                    k = w
                    while k > 1:
                        h = k // 2
                        nc.vector.tensor_tensor(
                            out=G[:, :, 0:h, :], in0=G[:, :, 0:h, :],
                            in1=G[:, :, h:2 * h, :], op=ALU.add)
                        if k % 2:
                            nc.vector.tensor_tensor(
                                out=G[:, :, 0:1, :], in0=G[:, :, 0:1, :],
                                in1=G[:, :, k - 1:k, :], op=ALU.add)
                        k = h
                    nc.vector.tensor_copy(
                        agg[:, g0:g0 + nw, :].unsqueeze(2), G[:, :, 0:1, :])
                # ---------------- normalize ----------------
                if DBG and l == 0:
                    nc.sync.dma_start(dbg_agg.ap(), agg[:])
                nc.vector.tensor_scalar(
                    out=denf[:], in0=agg[:, :, 32:34],
                    scalar1=float(inv_ks[l]), scalar2=1e-6,
                    op0=ALU.mult, op1=ALU.add)
                nc.vector.reciprocal(denf[:], denf[:])
                if l < 2:
                    nc.vector.tensor_tensor(
                        out=agg[:, :, 0:32].rearrange("p w (a b) -> p w a b", b=2),
                        in0=agg[:, :, 0:32].rearrange("p w (a b) -> p w a b", b=2),
                        in1=denf[:].unsqueeze(2).to_broadcast([P, NW, 16, 2]),
                        op=ALU.mult)
                else:
                    nc.vector.tensor_tensor(
                        out=outf[:].rearrange("p w (a b) -> p w a b", b=2),
                        in0=agg[:, :, 0:32].rearrange("p w (a b) -> p w a b", b=2),
                        in1=denf[:].unsqueeze(2).to_broadcast([P, NW, 16, 2]),
                        op=ALU.mult)
                    nc.vector.tensor_tensor(
                        out=outf[:], in0=outf[:],
                        in1=b3_t[:].unsqueeze(1).to_broadcast([P, NW, 32]),
                        op=ALU.add)
                    nc.sync.dma_start(out_d.ap().transpose([1, 0, 2]),
                                      outf[:])

    nc.compile()
    return nc


def kernel(x, edge_index, W1, att_s1, att_d1, b1, ea1,
           W2, att_s2, att_d2, b2, W3, att_s3, att_d3, b3):
    x = np.asarray(x, dtype=np.float32)
    Ws = [np.asarray(W1, np.float32), np.asarray(W2, np.float32),
          np.asarray(W3, np.float32)]
    att_ss = [np.asarray(att_s1, np.float32), np.asarray(att_s2, np.float32),
              np.asarray(att_s3, np.float32)]
    att_ds = [np.asarray(att_d1, np.float32), np.asarray(att_d2, np.float32),
              np.asarray(att_d3, np.float32)]
    bs = [np.asarray(b1, np.float32), np.asarray(b2, np.float32),
          np.asarray(b3, np.float32)]

    s = float(np.tanh(np.asarray(ea1, np.float32))[0])
    if s < 0.1:
        s = 1.0
    ks = [s * 1.05, 1.0, 1.0]
    Hs = [2, 2, 1]
    slopes = [0.01, 0.2, 0.2]

    cores, gw, ngw, gcol0, S, table_pos = _host_prep(edge_index)

    # fused weights [32 in, 36 out] per layer with interleave perms
    perms_out = [_interleave_nat(2), _interleave_nat(2), np.arange(32)]
    perms_in = [np.arange(32), _interleave_nat(2), _interleave_nat(2)]
    wbs = []
    for l in range(3):
        Wt = Ws[l].T  # [in, out] natural
        a_s, a_d = att_ss[l], att_ds[l]
        H = a_s.shape[0]
        CH = a_s.shape[1]
        M = np.zeros((32, 36), dtype=np.float32)
        M[:, 0:32] = Wt[np.ix_(perms_in[l], perms_out[l])]
        for j in range(2):
            h = j if H == 2 else 0
            M[:, 32 + j] = (Wt[:, h * CH:(h + 1) * CH] @ a_s[h])[perms_in[l]]
            M[:, 34 + j] = (Wt[:, h * CH:(h + 1) * CH] @ a_d[h])[perms_in[l]]
        wbs.append(M)
    wb_cat = np.concatenate(wbs, axis=1).astype(np.float16)

    b12 = np.concatenate([
        (ks[0] * bs[0])[perms_out[0]],
        (ks[1] * bs[1])[perms_out[1]],
    ]).astype(np.float16)
    b12_cat = np.tile(b12[None, :], (P, 1))
    b3_cat = np.tile(bs[2][None, :], (P, 1)).astype(np.float32)

    in_maps = []
    for c in range(NC):
        cc = cores[c]
        x_pad = np.zeros((NP, 32), dtype=np.float16)
        x_pad[:cc["n_loc"]] = x[cc["perm"]].astype(np.float16)
        in_maps.append({
            "x_sh": x_pad.reshape(NW, P, 32),
            "idx_in": cc["idx"],
            "wb_in": wb_cat,
            "b12_in": b12_cat,
            "b3_in": b3_cat,
        })

    nc = _build_program(gw, ngw, gcol0, S, slopes,
                        [1.0 / k for k in ks])
    global LAST_EXEC_NS
    try:
        from concourse.timeline_sim import TimelineSim
        LAST_EXEC_NS = TimelineSim(nc, no_exec=True).simulate()
    except Exception:
        LAST_EXEC_NS = None
    res = run_bass_kernel_spmd(nc, in_maps, list(range(NC)))

    out = np.empty((N_NODES, 32), dtype=np.float32)
    for c in range(NC):
        cc = cores[c]
        ob = res.results[c]["out_d"].reshape(NP, 32)
        out[cc["perm"]] = ob[:cc["n_loc"]]
    return out


# revision 11
# speedup vs baseline: 1.0229x; 1.0229x over previous
"""EnhancedRGCN (3-layer GAT) Trainium2 kernel, 8-core SPMD.

Sharding: destination nodes across 8 cores. Host builds a static padded-CSR
(dst-degree-sorted, windows of 128 dst nodes, groups of 4 windows sharing a
padded width). Per layer: node phase computes fp16 table rows
[h | a_s-pair | a_d-pair] = act(prev) @ Wbig via PE, one transposed-AP DMA
writes the shard table, AllGather exchanges shards, edge phase gathers rows
per group with a single batched indirect DMA (flat-table AP, descriptors
collapse to 128/call), then DVE/ACT run the segment softmax + weighted
aggregation in fp16 with a pair-interleaved head layout (keeps tensor_tensor
in 2x mode). Softmax max-subtraction is skipped (bounded logits); pad slots
gather a dedicated table row with h=0, a_s=-3000 so exp() underflows to 0.
"""

import sys

sys.path.insert(0, "/opt/trn_rl_repo")

import numpy as np

from concourse import bass, bacc, mybir, tile
from concourse.bass_utils import run_bass_kernel_spmd
from concourse.masks import make_identity

NC = 8
P = 128
GS = 4  # windows per group
F32 = mybir.dt.float32
F16 = mybir.dt.float16
I32 = mybir.dt.int32
ALU = mybir.AluOpType
ACTF = mybir.ActivationFunctionType

N_NODES = 100000
NPC = (N_NODES + NC - 1) // NC          # 12500
NW = (NPC + P - 1) // P                 # 98
NP = NW * P                             # 12544
TBL = NC * NP
NG = (NW + GS - 1) // GS                # 25 groups (24x4 + 1x2)
PAD_ROW = NP - 1                        # core 0's rank 12543, never used by data


def _host_prep(edge_index):
    src = np.asarray(edge_index[0], dtype=np.int64)
    dst = np.asarray(edge_index[1], dtype=np.int64)

    table_pos = np.empty(N_NODES, dtype=np.int64)
    perms = []
    dws = np.zeros((NC, NW), dtype=np.int64)
    cores = []
    for c in range(NC):
        lo, hi = c * NPC, min((c + 1) * NPC, N_NODES)
        n_loc = hi - lo
        emask = (dst >= lo) & (dst < hi)
        e_src, e_dst = src[emask], dst[emask] - lo
        deg = np.bincount(e_dst, minlength=n_loc)
        order = np.argsort(-deg, kind="stable")
        perms.append(order + lo)
        table_pos[order + lo] = c * NP + np.arange(n_loc)
        rank_of_local = np.empty(n_loc, dtype=np.int64)
        rank_of_local[order] = np.arange(n_loc)
        e_rank = rank_of_local[e_dst]
        deg_r = np.bincount(e_rank, minlength=NP)
        dws[c] = [max(int(deg_r[w * P:(w + 1) * P].max()), 1) for w in range(NW)]
        o = np.argsort(e_rank, kind="stable")
        e_rank_s, e_src_s = e_rank[o], e_src[o]
        slot = np.arange(len(e_rank_s)) - np.concatenate(
            [[0], np.cumsum(deg_r)])[e_rank_s]
        cores.append(dict(n_loc=n_loc, e_rank=e_rank_s, e_src=e_src_s,
                          slot=slot))

    dw_u = dws.max(axis=0)
    # group widths: max over the GS windows in each group (shared program)
    gw = np.array([int(dw_u[g * GS:(g + 1) * GS].max()) for g in range(NG)])
    ngw = np.array([min(GS, NW - g * GS) for g in range(NG)])
    gcol0 = np.concatenate([[0], np.cumsum(ngw * gw)]).astype(int)
    S = int(gcol0[-1])

    for c in range(NC):
        cc = cores[c]
        w_of = cc["e_rank"] // P
        row = cc["e_rank"] % P
        g_of = w_of // GS
        col = gcol0[g_of] + (w_of - g_of * GS) * gw[g_of] + cc["slot"]
        idx_u = np.full((P, S), PAD_ROW * 34, dtype=np.int32)
        idx_u[row, col] = (table_pos[cc["e_src"]] * 34).astype(np.int32)
        cc["idx"] = idx_u
        cc["perm"] = perms[c]
    return cores, gw, ngw, gcol0, S, table_pos, dw_u


def _interleave_nat(H):
    # natural feature index for each interleaved column j
    if H == 1:
        return np.arange(32)
    il = np.empty(32, dtype=np.int64)
    il[0::2] = np.arange(16)          # head 0
    il[1::2] = 16 + np.arange(16)     # head 1
    return il


def _build_program(gw, ngw, gcol0, S, slopes, inv_ks, dwu):
    nc = bacc.Bacc("TRN2", target_bir_lowering=False, debug=False,
                   num_devices=NC, dynamic_dma_scratch_size=65536)

    x_sh = nc.dram_tensor("x_sh", [NW, P, 32], F16, kind="ExternalInput")
    idx_in = nc.dram_tensor("idx_in", [P, S], I32, kind="ExternalInput")
    wb_in = nc.dram_tensor("wb_in", [32, 108], F16, kind="ExternalInput")
    b12_in = nc.dram_tensor("b12_in", [P, 64], F16, kind="ExternalInput")
    b3_in = nc.dram_tensor("b3_in", [P, 32], F32, kind="ExternalInput")
    out_d = nc.dram_tensor("out_d", [NW, P, 32], F32, kind="ExternalOutput")

    tbl_sh = nc.dram_tensor("tbl_sh", [NW, P, 34], F16)
    tbl_full = nc.dram_tensor("tbl_full", [1, TBL * 34], F16,
                              addr_space="Shared")
    import os
    DBG = os.environ.get("K_DEBUG", "") == "1"
    if DBG:
        dbg_xall = nc.dram_tensor("dbg_xall", [P, NW, 32], F16,
                                  kind="ExternalOutput")
        dbg_nodev = nc.dram_tensor("dbg_nodev", [P, NW, 36], F16,
                                   kind="ExternalOutput")
        dbg_G = nc.dram_tensor("dbg_G", [P, 4 * int(gw[0]), 34], F16,
                               kind="ExternalOutput")
        dbg_agg = nc.dram_tensor("dbg_agg", [P, NW, 34], F16,
                                 kind="ExternalOutput")

    with tile.TileContext(nc) as tc:
        with (
            tc.tile_pool(name="res", bufs=1) as res,
            tc.tile_pool(name="xTp", bufs=2) as xTp,
            tc.tile_pool(name="gat", bufs=3) as gat,
            tc.tile_pool(name="ptp", bufs=2, space="PSUM") as ptp,
            tc.tile_pool(name="pmm", bufs=2, space="PSUM") as pmm,
        ):
            ident = res.tile([P, P], F16)
            make_identity(nc, ident[:])
            idx_t = res.tile([P, S], I32)
            nc.sync.dma_start(idx_t[:], idx_in[:])
            wb_t = res.tile([32, 108], F16)
            nc.sync.dma_start(wb_t[:], wb_in[:])
            b12_t = res.tile([P, 64], F16)
            nc.sync.dma_start(b12_t[:], b12_in[:])
            b3_t = res.tile([P, 32], F32)
            nc.sync.dma_start(b3_t[:], b3_in[:])
            xall = res.tile([P, NW, 32], F16)
            nc.sync.dma_start(xall[:], x_sh.ap().transpose([1, 0, 2]))
            tneg = res.tile([P, NW, 32], F16)
            nodev = res.tile([P, NW, 36], F16)
            agg = res.tile([P, NW, 34], F16)
            denf = res.tile([P, NW, 2], F32)
            outf = res.tile([P, NW, 32], F32)

            for l in range(3):
                slope = float(slopes[l])
                # ---------------- node phase ----------------
                if l > 0:
                    bia = b12_t[:, 32 * (l - 1):32 * l].unsqueeze(1) \
                        .to_broadcast([P, NW, 32])
                    nc.vector.tensor_tensor(out=xall[:], in0=agg[:, :, 0:32],
                                            in1=bia, op=ALU.add)
                    nc.vector.tensor_scalar_min(tneg[:], xall[:], 0.0)
                    nc.scalar.activation(tneg[:], tneg[:], ACTF.Exp)
                    nc.vector.tensor_scalar_max(xall[:], xall[:], 0.0)
                    nc.vector.tensor_tensor(out=xall[:], in0=xall[:],
                                            in1=tneg[:], op=ALU.add)
                    nc.vector.tensor_scalar(out=xall[:], in0=xall[:],
                                            scalar1=-1.0, scalar2=3.0,
                                            op0=ALU.add, op1=ALU.min)
                    nc.vector.tensor_scalar_max(xall[:], xall[:], -3.0)
                for g in range(NG):
                    g0, nw = g * GS, int(ngw[g])
                    pt = ptp.tile([32, GS, P], F16, tag="pt")
                    for r in range(nw):
                        nc.tensor.transpose(
                            out=pt[:, r, :],
                            in_=xall[:, g0 + r, :], identity=ident[:])
                    xT = xTp.tile([32, GS, P], F16, tag="xT")
                    nc.vector.tensor_copy(xT[:, 0:nw, :], pt[:, 0:nw, :])
                    mm = pmm.tile([P, GS, 36], F32, tag="mm")
                    for r in range(nw):
                        nc.tensor.matmul(
                            mm[:, r, :], lhsT=xT[:, r, :],
                            rhs=wb_t[:, 36 * l:36 * l + 36],
                            start=True, stop=True)
                    nc.vector.tensor_copy(nodev[:, g0:g0 + nw, :],
                                          mm[:, 0:nw, :])
                # pad rows: h=0, a_s=-3000 (ranks 12512..12543 unused)
                nc.vector.memset(nodev[96:128, NW - 1, 0:32], 0.0)
                nc.vector.memset(nodev[96:128, NW - 1, 32:34], -3000.0)
                if DBG and l == 0:
                    nc.sync.dma_start(dbg_xall.ap(), xall[:])
                    nc.sync.dma_start(dbg_nodev.ap(), nodev[:])
                nc.sync.dma_start(tbl_sh.ap().transpose([1, 0, 2]),
                                  nodev[:, :, 0:34])
                # ---------------- exchange ----------------
                nc.gpsimd.collective_compute(
                    "AllGather", ALU.bypass,
                    replica_groups=[list(range(NC))],
                    ins=[tbl_sh.ap()], outs=[tbl_full.ap()],
                )
                # ---------------- edge phase ----------------
                for g in range(NG):
                    g0, nw, w = g * GS, int(ngw[g]), int(gw[g])
                    c0 = int(gcol0[g])
                    G = gat.tile([P, nw, w, 34], F16, tag="G")
                    Gv = G[:].rearrange("p a b f -> p (a b) f")
                    for wi in range(nw):
                        dwe = int(dwu[g0 + wi])
                        if dwe < w:
                            nc.vector.memset(G[:, wi, dwe:w, 0:32], 0.0)
                            nc.vector.memset(G[:, wi, dwe:w, 32:34], -3000.0)
                        for cc in range(dwe):
                            nc.gpsimd.indirect_dma_start(
                                out=Gv[:, wi * w + cc, :], out_offset=None,
                                in_=tbl_full[:],
                                in_offset=bass.IndirectOffsetOnAxis(
                                    ap=idx_t[:, c0 + wi * w + cc:
                                             c0 + wi * w + cc + 1], axis=1),
                            )
                    if DBG and l == 0 and g == 0:
                        nc.sync.dma_start(dbg_G.ap(), G[:].opt())
                    tv = G[:, :, :, 32:34]
                    nc.vector.tensor_tensor(
                        out=tv, in0=tv,
                        in1=nodev[:, g0:g0 + nw, 34:36].unsqueeze(2)
                            .to_broadcast([P, nw, w, 2]),
                        op=ALU.add)
                    nc.scalar.activation(tv, tv, ACTF.Lrelu, alpha=slope)
                    nc.scalar.activation(tv, tv, ACTF.Exp)
                    for wi in range(nw):
                        nc.vector.tensor_tensor(
                            out=G[:, wi, :, 0:32].rearrange("p w (a b) -> p w a b", b=2),
                            in0=G[:, wi, :, 0:32].rearrange("p w (a b) -> p w a b", b=2),
                            in1=G[:, wi, :, 32:34].unsqueeze(2)
                                .to_broadcast([P, w, 16, 2]),
                            op=ALU.mult)
                    k = w
                    while k > 1:
                        h = k // 2
                        nc.vector.tensor_tensor(
                            out=G[:, :, 0:h, :], in0=G[:, :, 0:h, :],
                            in1=G[:, :, h:2 * h, :], op=ALU.add)
                        if k % 2:
                            nc.vector.tensor_tensor(
                                out=G[:, :, 0:1, :], in0=G[:, :, 0:1, :],
                                in1=G[:, :, k - 1:k, :], op=ALU.add)
                        k = h
                    nc.vector.tensor_copy(
                        agg[:, g0:g0 + nw, :].unsqueeze(2), G[:, :, 0:1, :])
                # ---------------- normalize ----------------
                if DBG and l == 0:
                    nc.sync.dma_start(dbg_agg.ap(), agg[:])
                nc.vector.tensor_scalar(
                    out=denf[:], in0=agg[:, :, 32:34],
                    scalar1=float(inv_ks[l]), scalar2=1e-6,
                    op0=ALU.mult, op1=ALU.add)
                nc.vector.reciprocal(denf[:], denf[:])
                if l < 2:
                    nc.vector.tensor_tensor(
                        out=agg[:, :, 0:32].rearrange("p w (a b) -> p w a b", b=2),
                        in0=agg[:, :, 0:32].rearrange("p w (a b) -> p w a b", b=2),
                        in1=denf[:].unsqueeze(2).to_broadcast([P, NW, 16, 2]),
                        op=ALU.mult)
                else:
                    nc.vector.tensor_tensor(
                        out=outf[:].rearrange("p w (a b) -> p w a b", b=2),
                        in0=agg[:, :, 0:32].rearrange("p w (a b) -> p w a b", b=2),
                        in1=denf[:].unsqueeze(2).to_broadcast([P, NW, 16, 2]),
                        op=ALU.mult)
                    nc.vector.tensor_tensor(
                        out=outf[:], in0=outf[:],
                        in1=b3_t[:].unsqueeze(1).to_broadcast([P, NW, 32]),
                        op=ALU.add)
                    nc.sync.dma_start(out_d.ap().transpose([1, 0, 2]),
                                      outf[:])

    nc.compile()
    return nc


def kernel(x, edge_index, W1, att_s1, att_d1, b1, ea1,
           W2, att_s2, att_d2, b2, W3, att_s3, att_d3, b3):
    x = np.asarray(x, dtype=np.float32)
    Ws = [np.asarray(W1, np.float32), np.asarray(W2, np.float32),
          np.asarray(W3, np.float32)]
    att_ss = [np.asarray(att_s1, np.float32), np.asarray(att_s2, np.float32),
              np.asarray(att_s3, np.float32)]
    att_ds = [np.asarray(att_d1, np.float32), np.asarray(att_d2, np.float32),
              np.asarray(att_d3, np.float32)]
    bs = [np.asarray(b1, np.float32), np.asarray(b2, np.float32),
          np.asarray(b3, np.float32)]

    s = float(np.tanh(np.asarray(ea1, np.float32))[0])
    if s < 0.1:
        s = 1.0
    ks = [s * 1.05, 1.0, 1.0]
    Hs = [2, 2, 1]
    slopes = [0.01, 0.2, 0.2]

    cores, gw, ngw, gcol0, S, table_pos, dw_u = _host_prep(edge_index)

    # fused weights [32 in, 36 out] per layer with interleave perms
    perms_out = [_interleave_nat(2), _interleave_nat(2), np.arange(32)]
    perms_in = [np.arange(32), _interleave_nat(2), _interleave_nat(2)]
    wbs = []
    for l in range(3):
        Wt = Ws[l].T  # [in, out] natural
        a_s, a_d = att_ss[l], att_ds[l]
        H = a_s.shape[0]
        CH = a_s.shape[1]
        M = np.zeros((32, 36), dtype=np.float32)
        M[:, 0:32] = Wt[np.ix_(perms_in[l], perms_out[l])]
        for j in range(2):
            h = j if H == 2 else 0
            M[:, 32 + j] = (Wt[:, h * CH:(h + 1) * CH] @ a_s[h])[perms_in[l]]
            M[:, 34 + j] = (Wt[:, h * CH:(h + 1) * CH] @ a_d[h])[perms_in[l]]
        wbs.append(M)
    wb_cat = np.concatenate(wbs, axis=1).astype(np.float16)

    b12 = np.concatenate([
        (ks[0] * bs[0])[perms_out[0]],
        (ks[1] * bs[1])[perms_out[1]],
    ]).astype(np.float16)
    b12_cat = np.tile(b12[None, :], (P, 1))
    b3_cat = np.tile(bs[2][None, :], (P, 1)).astype(np.float32)

    in_maps = []
    for c in range(NC):
        cc = cores[c]
        x_pad = np.zeros((NP, 32), dtype=np.float16)
        x_pad[:cc["n_loc"]] = x[cc["perm"]].astype(np.float16)
        in_maps.append({
            "x_sh": x_pad.reshape(NW, P, 32),
            "idx_in": cc["idx"],
            "wb_in": wb_cat,
            "b12_in": b12_cat,
            "b3_in": b3_cat,
        })

    nc = _build_program(gw, ngw, gcol0, S, slopes,
                        [1.0 / k for k in ks], dw_u)
    global LAST_EXEC_NS
    try:
        from concourse.timeline_sim import TimelineSim
        LAST_EXEC_NS = TimelineSim(nc, no_exec=True).simulate()
    except Exception:
        LAST_EXEC_NS = None
    res = run_bass_kernel_spmd(nc, in_maps, list(range(NC)))

    out = np.empty((N_NODES, 32), dtype=np.float32)
    for c in range(NC):
        cc = cores[c]
        ob = res.results[c]["out_d"].reshape(NP, 32)
        out[cc["perm"]] = ob[:cc["n_loc"]]
    return out


# revision 12
# speedup vs baseline: 1.0294x; 1.0064x over previous
"""EnhancedRGCN (3-layer GAT) Trainium2 kernel, 8-core SPMD.

Sharding: destination nodes across 8 cores. Host builds a static padded-CSR
(dst-degree-sorted, windows of 128 dst nodes, groups of 4 windows sharing a
padded width). Per layer: node phase computes fp16 table rows
[h | a_s-pair | a_d-pair] = act(prev) @ Wbig via PE, one transposed-AP DMA
writes the shard table, AllGather exchanges shards, edge phase gathers rows
per group with a single batched indirect DMA (flat-table AP, descriptors
collapse to 128/call), then DVE/ACT run the segment softmax + weighted
aggregation in fp16 with a pair-interleaved head layout (keeps tensor_tensor
in 2x mode). Softmax max-subtraction is skipped (bounded logits); pad slots
gather a dedicated table row with h=0, a_s=-3000 so exp() underflows to 0.
"""

import sys

sys.path.insert(0, "/opt/trn_rl_repo")

import numpy as np

from concourse import bass, bacc, mybir, tile
from concourse.bass_utils import run_bass_kernel_spmd
from concourse.masks import make_identity

NC = 8
P = 128
GS = 4  # windows per group
F32 = mybir.dt.float32
F16 = mybir.dt.float16
I32 = mybir.dt.int32
ALU = mybir.AluOpType
ACTF = mybir.ActivationFunctionType

N_NODES = 100000
NPC = (N_NODES + NC - 1) // NC          # 12500
NW = (NPC + P - 1) // P                 # 98
NP = NW * P                             # 12544
TBL = NC * NP
NG = (NW + GS - 1) // GS                # 25 groups (24x4 + 1x2)
PAD_ROW = NP - 1                        # core 0's rank 12543, never used by data


def _host_prep(edge_index):
    src = np.asarray(edge_index[0], dtype=np.int64)
    dst = np.asarray(edge_index[1], dtype=np.int64)

    table_pos = np.empty(N_NODES, dtype=np.int64)
    perms = []
    dws = np.zeros((NC, NW), dtype=np.int64)
    cores = []
    for c in range(NC):
        lo, hi = c * NPC, min((c + 1) * NPC, N_NODES)
        n_loc = hi - lo
        emask = (dst >= lo) & (dst < hi)
        e_src, e_dst = src[emask], dst[emask] - lo
        deg = np.bincount(e_dst, minlength=n_loc)
        order = np.argsort(-deg, kind="stable")
        perms.append(order + lo)
        table_pos[order + lo] = c * NP + np.arange(n_loc)
        rank_of_local = np.empty(n_loc, dtype=np.int64)
        rank_of_local[order] = np.arange(n_loc)
        e_rank = rank_of_local[e_dst]
        deg_r = np.bincount(e_rank, minlength=NP)
        dws[c] = [max(int(deg_r[w * P:(w + 1) * P].max()), 1) for w in range(NW)]
        o = np.argsort(e_rank, kind="stable")
        e_rank_s, e_src_s = e_rank[o], e_src[o]
        slot = np.arange(len(e_rank_s)) - np.concatenate(
            [[0], np.cumsum(deg_r)])[e_rank_s]
        cores.append(dict(n_loc=n_loc, e_rank=e_rank_s, e_src=e_src_s,
                          slot=slot))

    dw_u = dws.max(axis=0)
    # group widths: max over the GS windows in each group (shared program)
    gw = np.array([int(dw_u[g * GS:(g + 1) * GS].max()) for g in range(NG)])
    ngw = np.array([min(GS, NW - g * GS) for g in range(NG)])
    gcol0 = np.concatenate([[0], np.cumsum(ngw * gw)]).astype(int)
    S = int(gcol0[-1])

    for c in range(NC):
        cc = cores[c]
        w_of = cc["e_rank"] // P
        row = cc["e_rank"] % P
        g_of = w_of // GS
        col = gcol0[g_of] + (w_of - g_of * GS) * gw[g_of] + cc["slot"]
        idx_u = np.full((P, S), PAD_ROW * 34, dtype=np.int32)
        idx_u[row, col] = (table_pos[cc["e_src"]] * 34).astype(np.int32)
        cc["idx"] = idx_u
        cc["perm"] = perms[c]
    return cores, gw, ngw, gcol0, S, table_pos, dw_u


def _interleave_nat(H):
    # natural feature index for each interleaved column j
    if H == 1:
        return np.arange(32)
    il = np.empty(32, dtype=np.int64)
    il[0::2] = np.arange(16)          # head 0
    il[1::2] = 16 + np.arange(16)     # head 1
    return il


def _build_program(gw, ngw, gcol0, S, slopes, inv_ks, dwu):
    nc = bacc.Bacc("TRN2", target_bir_lowering=False, debug=False,
                   num_devices=NC, dynamic_dma_scratch_size=65536)

    x_sh = nc.dram_tensor("x_sh", [NW, P, 32], F16, kind="ExternalInput")
    idx_in = nc.dram_tensor("idx_in", [P, S], I32, kind="ExternalInput")
    wb_in = nc.dram_tensor("wb_in", [32, 108], F16, kind="ExternalInput")
    b12_in = nc.dram_tensor("b12_in", [P, 64], F16, kind="ExternalInput")
    b3_in = nc.dram_tensor("b3_in", [P, 32], F32, kind="ExternalInput")
    out_d = nc.dram_tensor("out_d", [NW, P, 32], F32, kind="ExternalOutput")

    tbl_sh = nc.dram_tensor("tbl_sh", [NW, P, 34], F16)
    tbl_full = nc.dram_tensor("tbl_full", [1, TBL * 34], F16,
                              addr_space="Shared")
    import os
    DBG = os.environ.get("K_DEBUG", "") == "1"
    if DBG:
        dbg_xall = nc.dram_tensor("dbg_xall", [P, NW, 32], F16,
                                  kind="ExternalOutput")
        dbg_nodev = nc.dram_tensor("dbg_nodev", [P, NW, 36], F16,
                                   kind="ExternalOutput")
        dbg_G = nc.dram_tensor("dbg_G", [P, 4 * int(gw[0]), 34], F16,
                               kind="ExternalOutput")
        dbg_agg = nc.dram_tensor("dbg_agg", [P, NW, 34], F16,
                                 kind="ExternalOutput")

    with tile.TileContext(nc) as tc:
        with (
            tc.tile_pool(name="res", bufs=1) as res,
            tc.tile_pool(name="xTp", bufs=2) as xTp,
            tc.tile_pool(name="gat", bufs=3) as gat,
            tc.tile_pool(name="ptp", bufs=2, space="PSUM") as ptp,
            tc.tile_pool(name="pmm", bufs=2, space="PSUM") as pmm,
        ):
            ident = res.tile([P, P], F16)
            make_identity(nc, ident[:])
            idx_t = res.tile([P, S], I32)
            nc.sync.dma_start(idx_t[:], idx_in[:])
            wb_t = res.tile([32, 108], F16)
            nc.sync.dma_start(wb_t[:], wb_in[:])
            b12_t = res.tile([P, 64], F16)
            nc.sync.dma_start(b12_t[:], b12_in[:])
            b3_t = res.tile([P, 32], F32)
            nc.sync.dma_start(b3_t[:], b3_in[:])
            xall = res.tile([P, NW, 32], F16)
            nc.sync.dma_start(xall[:], x_sh.ap().transpose([1, 0, 2]))
            tneg = res.tile([P, NW, 32], F16)
            nodev_a = res.tile([P, NW, 36], F16)
            nodev_b = res.tile([P, NW, 36], F16)
            agg = res.tile([P, NW, 34], F16)
            denf = res.tile([P, NW, 2], F32)
            outf = res.tile([P, NW, 32], F32)

            def node_group(l, g, ndv):
                g0, nw = g * GS, int(ngw[g])
                if l > 0:
                    sl = slice(g0, g0 + nw)
                    bia = b12_t[:, 32 * (l - 1):32 * l].unsqueeze(1) \
                        .to_broadcast([P, nw, 32])
                    nc.vector.tensor_tensor(out=xall[:, sl, :],
                                            in0=agg[:, sl, 0:32],
                                            in1=bia, op=ALU.add)
                    nc.vector.tensor_scalar_min(tneg[:, sl, :],
                                                xall[:, sl, :], 0.0)
                    nc.scalar.activation(tneg[:, sl, :], tneg[:, sl, :],
                                         ACTF.Exp)
                    nc.vector.tensor_scalar_max(xall[:, sl, :],
                                                xall[:, sl, :], 0.0)
                    nc.vector.tensor_tensor(out=xall[:, sl, :],
                                            in0=xall[:, sl, :],
                                            in1=tneg[:, sl, :], op=ALU.add)
                    nc.vector.tensor_scalar(out=xall[:, sl, :],
                                            in0=xall[:, sl, :],
                                            scalar1=-1.0, scalar2=3.0,
                                            op0=ALU.add, op1=ALU.min)
                    nc.vector.tensor_scalar_max(xall[:, sl, :],
                                                xall[:, sl, :], -3.0)
                pt = ptp.tile([32, GS, P], F16, tag="pt")
                for r in range(nw):
                    nc.tensor.transpose(
                        out=pt[:, r, :],
                        in_=xall[:, g0 + r, :], identity=ident[:])
                xT = xTp.tile([32, GS, P], F16, tag="xT")
                nc.vector.tensor_copy(xT[:, 0:nw, :], pt[:, 0:nw, :])
                mm = pmm.tile([P, GS, 36], F32, tag="mm")
                for r in range(nw):
                    nc.tensor.matmul(
                        mm[:, r, :], lhsT=xT[:, r, :],
                        rhs=wb_t[:, 36 * l:36 * l + 36],
                        start=True, stop=True)
                nc.vector.tensor_copy(ndv[:, g0:g0 + nw, :], mm[:, 0:nw, :])

            def finish_table(ndv):
                nc.vector.memset(ndv[96:128, NW - 1, 0:32], 0.0)
                nc.vector.memset(ndv[96:128, NW - 1, 32:34], -3000.0)
                nc.sync.dma_start(tbl_sh.ap().transpose([1, 0, 2]),
                                  ndv[:, :, 0:34])

            for g in range(NG):
                node_group(0, g, nodev_a)
            finish_table(nodev_a)

            for l in range(3):
                slope = float(slopes[l])
                nodev = nodev_a if l % 2 == 0 else nodev_b
                nodev_nxt = nodev_b if l % 2 == 0 else nodev_a
                nc.gpsimd.collective_compute(
                    "AllGather", ALU.bypass,
                    replica_groups=[list(range(NC))],
                    ins=[tbl_sh.ap()], outs=[tbl_full.ap()],
                )
                for g in range(NG):
                    g0, nw, w = g * GS, int(ngw[g]), int(gw[g])
                    c0 = int(gcol0[g])
                    G = gat.tile([P, nw, w, 34], F16, tag="G")
                    Gv = G[:].rearrange("p a b f -> p (a b) f")
                    for wi in range(nw):
                        dwe = int(dwu[g0 + wi])
                        if dwe < w:
                            nc.vector.memset(G[:, wi, dwe:w, 0:32], 0.0)
                            nc.vector.memset(G[:, wi, dwe:w, 32:34], -3000.0)
                        for cc in range(dwe):
                            nc.gpsimd.indirect_dma_start(
                                out=Gv[:, wi * w + cc, :], out_offset=None,
                                in_=tbl_full[:],
                                in_offset=bass.IndirectOffsetOnAxis(
                                    ap=idx_t[:, c0 + wi * w + cc:
                                             c0 + wi * w + cc + 1], axis=1),
                            )
                    tv = G[:, :, :, 32:34]
                    nc.vector.tensor_tensor(
                        out=tv, in0=tv,
                        in1=nodev[:, g0:g0 + nw, 34:36].unsqueeze(2)
                            .to_broadcast([P, nw, w, 2]),
                        op=ALU.add)
                    nc.scalar.activation(tv, tv, ACTF.Lrelu, alpha=slope)
                    nc.scalar.activation(tv, tv, ACTF.Exp)
                    for wi in range(nw):
                        nc.vector.tensor_tensor(
                            out=G[:, wi, :, 0:32].rearrange("p w (a b) -> p w a b", b=2),
                            in0=G[:, wi, :, 0:32].rearrange("p w (a b) -> p w a b", b=2),
                            in1=G[:, wi, :, 32:34].unsqueeze(2)
                                .to_broadcast([P, w, 16, 2]),
                            op=ALU.mult)
                    k = w
                    while k > 1:
                        h = k // 2
                        nc.vector.tensor_tensor(
                            out=G[:, :, 0:h, :], in0=G[:, :, 0:h, :],
                            in1=G[:, :, h:2 * h, :], op=ALU.add)
                        if k % 2:
                            nc.vector.tensor_tensor(
                                out=G[:, :, 0:1, :], in0=G[:, :, 0:1, :],
                                in1=G[:, :, k - 1:k, :], op=ALU.add)
                        k = h
                    nc.vector.tensor_copy(
                        agg[:, g0:g0 + nw, :].unsqueeze(2), G[:, :, 0:1, :])
                    # per-group normalize + (for l<2) next-layer node phase
                    gsl = slice(g0, g0 + nw)
                    nc.vector.tensor_scalar(
                        out=denf[:, gsl, :], in0=agg[:, gsl, 32:34],
                        scalar1=float(inv_ks[l]), scalar2=1e-6,
                        op0=ALU.mult, op1=ALU.add)
                    nc.vector.reciprocal(denf[:, gsl, :], denf[:, gsl, :])
                    if l < 2:
                        nc.vector.tensor_tensor(
                            out=agg[:, gsl, 0:32].rearrange("p w (a b) -> p w a b", b=2),
                            in0=agg[:, gsl, 0:32].rearrange("p w (a b) -> p w a b", b=2),
                            in1=denf[:, gsl, :].unsqueeze(2)
                                .to_broadcast([P, nw, 16, 2]),
                            op=ALU.mult)
                        node_group(l + 1, g, nodev_nxt)
                    else:
                        nc.vector.tensor_tensor(
                            out=outf[:, gsl, :].rearrange("p w (a b) -> p w a b", b=2),
                            in0=agg[:, gsl, 0:32].rearrange("p w (a b) -> p w a b", b=2),
                            in1=denf[:, gsl, :].unsqueeze(2)
                                .to_broadcast([P, nw, 16, 2]),
                            op=ALU.mult)
                        nc.vector.tensor_tensor(
                            out=outf[:, gsl, :], in0=outf[:, gsl, :],
                            in1=b3_t[:].unsqueeze(1).to_broadcast([P, nw, 32]),
                            op=ALU.add)
                if l < 2:
                    finish_table(nodev_nxt)
            nc.sync.dma_start(out_d.ap().transpose([1, 0, 2]), outf[:])

    nc.compile()
    return nc


def kernel(x, edge_index, W1, att_s1, att_d1, b1, ea1,
           W2, att_s2, att_d2, b2, W3, att_s3, att_d3, b3):
    x = np.asarray(x, dtype=np.float32)
    Ws = [np.asarray(W1, np.float32), np.asarray(W2, np.float32),
          np.asarray(W3, np.float32)]
    att_ss = [np.asarray(att_s1, np.float32), np.asarray(att_s2, np.float32),
              np.asarray(att_s3, np.float32)]
    att_ds = [np.asarray(att_d1, np.float32), np.asarray(att_d2, np.float32),
              np.asarray(att_d3, np.float32)]
    bs = [np.asarray(b1, np.float32), np.asarray(b2, np.float32),
          np.asarray(b3, np.float32)]

    s = float(np.tanh(np.asarray(ea1, np.float32))[0])
    if s < 0.1:
        s = 1.0
    ks = [s * 1.05, 1.0, 1.0]
    Hs = [2, 2, 1]
    slopes = [0.01, 0.2, 0.2]

    cores, gw, ngw, gcol0, S, table_pos, dw_u = _host_prep(edge_index)

    # fused weights [32 in, 36 out] per layer with interleave perms
    perms_out = [_interleave_nat(2), _interleave_nat(2), np.arange(32)]
    perms_in = [np.arange(32), _interleave_nat(2), _interleave_nat(2)]
    wbs = []
    for l in range(3):
        Wt = Ws[l].T  # [in, out] natural
        a_s, a_d = att_ss[l], att_ds[l]
        H = a_s.shape[0]
        CH = a_s.shape[1]
        M = np.zeros((32, 36), dtype=np.float32)
        M[:, 0:32] = Wt[np.ix_(perms_in[l], perms_out[l])]
        for j in range(2):
            h = j if H == 2 else 0
            M[:, 32 + j] = (Wt[:, h * CH:(h + 1) * CH] @ a_s[h])[perms_in[l]]
            M[:, 34 + j] = (Wt[:, h * CH:(h + 1) * CH] @ a_d[h])[perms_in[l]]
        wbs.append(M)
    wb_cat = np.concatenate(wbs, axis=1).astype(np.float16)

    b12 = np.concatenate([
        (ks[0] * bs[0])[perms_out[0]],
        (ks[1] * bs[1])[perms_out[1]],
    ]).astype(np.float16)
    b12_cat = np.tile(b12[None, :], (P, 1))
    b3_cat = np.tile(bs[2][None, :], (P, 1)).astype(np.float32)

    in_maps = []
    for c in range(NC):
        cc = cores[c]
        x_pad = np.zeros((NP, 32), dtype=np.float16)
        x_pad[:cc["n_loc"]] = x[cc["perm"]].astype(np.float16)
        in_maps.append({
            "x_sh": x_pad.reshape(NW, P, 32),
            "idx_in": cc["idx"],
            "wb_in": wb_cat,
            "b12_in": b12_cat,
            "b3_in": b3_cat,
        })

    nc = _build_program(gw, ngw, gcol0, S, slopes,
                        [1.0 / k for k in ks], dw_u)
    global LAST_EXEC_NS
    try:
        from concourse.timeline_sim import TimelineSim
        LAST_EXEC_NS = TimelineSim(nc, no_exec=True).simulate()
    except Exception:
        LAST_EXEC_NS = None
    res = run_bass_kernel_spmd(nc, in_maps, list(range(NC)))

    out = np.empty((N_NODES, 32), dtype=np.float32)
    for c in range(NC):
        cc = cores[c]
        ob = res.results[c]["out_d"].reshape(NP, 32)
        out[cc["perm"]] = ob[:cc["n_loc"]]
    return out
